# revision 11
# baseline (speedup 1.0000x reference)
"""Row-wise argmax + label lookup kernel for Trainium2 (8 NeuronCores).

Problem: inputs (16777216, 8) f32, label_table (8,) int32.
    y[i] = label_table[argmax(inputs[i, :])]   (first-occurrence ties)

Sharding: rows split evenly across 8 cores (data parallel, no comms).
Each core streams its 64 MiB slice through SBUF in 2 MiB tiles and
computes a grouped (groups of 8 along the free dim) first-occurrence
argmax per row on the Vector engine. The tiny label lookup is applied
on the host with the int row-argmax from the device.
"""

import numpy as np

N_CORES = 8
ROWS = 16777216
C = 8
ROWS_PER_CORE = ROWS // N_CORES  # 2_097_152
P = 128
TILE_F = 4096  # f32 elements per partition per tile (16 KiB)
GROUPS = TILE_F // C  # 512 rows per partition per tile
ROWS_PER_TILE = P * GROUPS  # 65_536
N_TILES = ROWS_PER_CORE // ROWS_PER_TILE  # 32

_NC_CACHE = {}


def _build_nc(n_tiles=N_TILES):
    import concourse.tile as tile
    from concourse import bacc, mybir

    f32 = mybir.dt.float32
    i32 = mybir.dt.int32
    Alu = mybir.AluOpType

    rows = n_tiles * ROWS_PER_TILE
    nc = bacc.Bacc("TRN2", target_bir_lowering=False)
    x = nc.dram_tensor("x", [rows * C], f32, kind="ExternalInput")
    y = nc.dram_tensor("y", [rows], i32, kind="ExternalOutput")
    xr = x.rearrange("(t p f) -> t p f", t=n_tiles, p=P)
    yr = y.rearrange("(t p j) -> t p j", t=n_tiles, p=P)

    with tile.TileContext(nc) as tc:
        with tc.tile_pool(name="xp", bufs=3) as xp, \
             tc.tile_pool(name="ep", bufs=2) as ep, \
             tc.tile_pool(name="sp", bufs=4) as sp, \
             tc.tile_pool(name="op", bufs=3) as op, \
             tc.tile_pool(name="cst", bufs=1) as cst:
            # w[:, c] = 7 - c  (descending weights; first max wins the
            # weighted reduce_max below)
            w = cst.tile([P, C], f32)
            for c in range(C):
                nc.vector.memset(w[:, c:c + 1], float(C - 1 - c))

            for t in range(n_tiles):
                xt = xp.tile([P, TILE_F], f32)
                nc.gpsimd.dma_start(out=xt[:], in_=xr[t])
                x3 = xt[:].rearrange("p (j c) -> p j c", c=C)

                # m[p, j] = max_c x[p, j, c]
                m = sp.tile([P, GROUPS], f32, tag="m")
                nc.vector.reduce_max(out=m[:], in_=x3, axis=mybir.AxisListType.X)
                mb = m[:].unsqueeze(2).broadcast_to([P, GROUPS, C])

                # eq = (x >= m) * w  -> grouped reduce_max gives 7 - argmax
                eq = ep.tile([P, TILE_F], f32)
                e3 = eq[:].rearrange("p (j c) -> p j c", c=C)
                nc.vector.tensor_tensor(out=e3, in0=x3, in1=mb, op=Alu.is_ge)
                wb = w[:].unsqueeze(1).broadcast_to([P, GROUPS, C])
                nc.vector.tensor_tensor(out=e3, in0=e3, in1=wb, op=Alu.mult)

                q = sp.tile([P, GROUPS], f32, tag="q")
                nc.vector.reduce_max(out=q[:], in_=e3, axis=mybir.AxisListType.X)

                # idx = 7 - q, cast to int32 on the output path
                o = op.tile([P, GROUPS], i32)
                nc.vector.tensor_scalar(
                    out=o[:], in0=q[:], scalar1=-1.0, scalar2=float(C - 1),
                    op0=Alu.mult, op1=Alu.add,
                )
                nc.gpsimd.dma_start(out=yr[t], in_=o[:])
    nc.finalize()
    return nc


def _get_nc(n_tiles=N_TILES):
    if n_tiles not in _NC_CACHE:
        _NC_CACHE[n_tiles] = _build_nc(n_tiles)
    return _NC_CACHE[n_tiles]


def kernel(inputs, label_table):
    x = np.ascontiguousarray(np.asarray(inputs, dtype=np.float32))
    lt = np.asarray(label_table)
    assert x.shape == (ROWS, C), x.shape

    from concourse.bass_utils import run_bass_kernel_spmd

    nc = _get_nc()
    in_maps = [
        {"x": x[i * ROWS_PER_CORE:(i + 1) * ROWS_PER_CORE].reshape(-1)}
        for i in range(N_CORES)
    ]
    res = run_bass_kernel_spmd(nc, in_maps, core_ids=list(range(N_CORES)))
    idx = np.concatenate([res.results[i]["y"] for i in range(N_CORES)])
    return np.take(lt, idx).astype(lt.dtype)


# revision 12
# speedup vs baseline: 1.8284x; 1.8284x over previous
"""Row-wise argmax + label lookup kernel for Trainium2 (8 NeuronCores).

Problem: inputs (16777216, 8) f32, label_table (8,) int32.
    y[i] = label_table[argmax(inputs[i, :])]   (first-occurrence ties)

Sharding: rows split evenly across 8 cores (data parallel, no comms).
Each core streams its 64 MiB slice through SBUF in 2 MiB tiles. Per tile
the Vector engine does:
  1. a pairwise tensor_tensor max tree (8 -> 4 -> 2 -> 1) for the row max
     (reads both operands on separate ports: 2048+1024+512 cycles), and
  2. one custom DVE instruction (GROUP_ARGMAX_ANT) that encodes the
     first-occurrence argmax of every 8-element group in a single 4096-cycle
     streaming pass, using a page-stepped score scan and a global running
     max over candidate scores (pages occupy disjoint ascending score
     ranges, so the running max self-segments), and
  3. a small tensor_tensor subtract against a static iota tile to decode
     the argmax of each row from the last element of its page.
The tiny label lookup is applied on the host from the int row-argmax.
"""

import numpy as np

N_CORES = 8
ROWS = 16777216
C = 8
ROWS_PER_CORE = ROWS // N_CORES  # 2_097_152
P = 128
TILE_F = 4096  # f32 elements per partition per tile (16 KiB)
GROUPS = TILE_F // C  # 512 rows per partition per tile
ROWS_PER_TILE = P * GROUPS  # 65_536
N_TILES = ROWS_PER_CORE // ROWS_PER_TILE  # 32

PG_STEP = 16.0  # custom-op page score step (2*C)

_NC_CACHE = {}
_REGISTERED = {}


# --------------------------------------------------------------------------
# Custom DVE op: grouped (pages of 8) first-occurrence argmax encoder.
#
#   Idx'  = scan(ADD, One, init=One)          -> g + 2  (g = global elem pos)
#   pg    = scan(ADD, One, init=C1, step=C1)  -> C1*(s+1)    (C1 = 16)
#   score = pg - Idx' = 8s + 14 - k           (k = pos in page, 0..7)
#   q     = Src0 >= Src1                      (element equals its group max)
#   cand  = q * score                         (0 when not a candidate)
#   run   = scan(MAX, cand, init=One)
#
# score is positive, strictly descending within a page, and pages occupy
# disjoint ascending ranges, so the global running max at the last element
# of page s equals the score of the FIRST group-max occurrence in page s:
#   run[s, 7] = 8s + 14 - argmax_s
# --------------------------------------------------------------------------

def _group_argmax_ref(in0, in1, s0, s1, imm2):
    a = np.asarray(in0, np.float32)
    Pp, S, N = a.shape
    b = np.broadcast_to(np.asarray(in1, np.float32), a.shape)
    s_idx = np.arange(S, dtype=np.float32)[None, :, None]
    k_idx = np.arange(N, dtype=np.float32)[None, None, :]
    idxp = s_idx * N + k_idx + 2.0
    pg = PG_STEP * (s_idx + 1.0)
    score = np.broadcast_to(pg - idxp, a.shape).astype(np.float32)
    q = (a >= b).astype(np.float32)
    cand = (q * score).astype(np.float32)
    run = np.maximum.accumulate(cand.reshape(Pp, S * N), axis=1).reshape(a.shape)
    return np.maximum(run, 1.0).astype(np.float32)  # scan init = One


def _get_group_argmax_op():
    if "op" in _REGISTERED:
        return _REGISTERED["op"]

    from concourse import dve_ops
    from concourse.dve_ops import DveOp
    from concourse.dve_spec import (
        C1, AluOp, One, Scan, Spec, Src0, Src1, lower,
    )
    from concourse.dve_uop import DveOpSpec

    name = "GROUP_ARGMAX_ANT"
    idxp = Scan(AluOp.ADD, One, init=One)
    pg = Scan(AluOp.ADD, One, init=C1, _subdim_step=C1)
    score = pg - idxp
    q = Src0 >= Src1
    cand = q * score
    # Scan.__post_init__ conservatively rejects scan-valued exprs, but the
    # scheduler resolves scheduled-Alu operands fine (each scan gets its own
    # stage with same-stage feedback). Swap the real expr in post-hoc.
    run = Scan(AluOp.MAX, One, init=One)
    object.__setattr__(run, "expr", cand)
    spec = Spec(body=run, reference=_group_argmax_ref)

    opcode = dve_ops._CUSTOM_DVE_ROW_BASE + len(dve_ops.OPS)
    assert opcode < 0x20
    dve_ops._SUB_OPCODE_FOR_NAME[name] = opcode
    shas = {}
    for ver in ("v3", "v4"):
        uops = lower(spec, ver=ver)
        shas[ver] = DveOpSpec(
            name=name, uops=uops, opcode=opcode, rd1_en=True
        ).sha(ver)
    op = DveOp(name, spec, subdim=True, uops_sha=shas)
    dve_ops.OPS.append(op)
    dve_ops.CUSTOM_DVE_SPECS[name] = spec
    _REGISTERED["op"] = op
    return op


def _build_nc(n_tiles=N_TILES):
    import concourse.tile as tile
    from concourse import bacc, mybir

    f32 = mybir.dt.float32
    i32 = mybir.dt.int32
    Alu = mybir.AluOpType
    argmax_op = _get_group_argmax_op()

    rows = n_tiles * ROWS_PER_TILE
    nc = bacc.Bacc("TRN2", target_bir_lowering=False)
    x = nc.dram_tensor("x", [rows * C], f32, kind="ExternalInput")
    y = nc.dram_tensor("y", [rows], i32, kind="ExternalOutput")
    xr = x.rearrange("(t p f) -> t p f", t=n_tiles, p=P)
    yr = y.rearrange("(t p j) -> t p j", t=n_tiles, p=P)

    with tile.TileContext(nc) as tc:
        with tc.tile_pool(name="xp", bufs=3) as xp, \
             tc.tile_pool(name="rp", bufs=2) as rp, \
             tc.tile_pool(name="mp", bufs=2) as mp, \
             tc.tile_pool(name="op_", bufs=3) as op_, \
             tc.tile_pool(name="cst", bufs=1) as cst:
            # i87[p, j] = 8j + 14 (decode tile for the custom op's scores)
            i87 = cst.tile([P, GROUPS], f32)
            nc.gpsimd.iota(i87[:, :], [[8, GROUPS]], channel_multiplier=0,
                           allow_small_or_imprecise_dtypes=True)
            nc.vector.tensor_scalar_add(i87[:, :], i87[:, :], 14.0)

            for t in range(n_tiles):
                xt = xp.tile([P, TILE_F], f32)
                nc.gpsimd.dma_start(out=xt[:], in_=xr[t])
                x3 = xt[:].rearrange("p (j c) -> p j c", c=C)

                # pairwise max tree: 8 -> 4 -> 2 -> 1 (both DVE read ports)
                m1 = mp.tile([P, GROUPS * 4], f32, tag="m1")
                m13 = m1[:].rearrange("p (j c) -> p j c", c=4)
                nc.vector.tensor_tensor(
                    out=m13, in0=x3[:, :, 0:8:2], in1=x3[:, :, 1:8:2], op=Alu.max)
                m2 = mp.tile([P, GROUPS * 2], f32, tag="m2")
                m23 = m2[:].rearrange("p (j c) -> p j c", c=2)
                nc.vector.tensor_tensor(
                    out=m23, in0=m13[:, :, 0:4:2], in1=m13[:, :, 1:4:2], op=Alu.max)
                m = mp.tile([P, GROUPS], f32, tag="m")
                nc.vector.tensor_tensor(
                    out=m[:].unsqueeze(2), in0=m23[:, :, 0:2:2],
                    in1=m23[:, :, 1:2:2], op=Alu.max)

                # one streaming pass encodes all grouped argmaxes
                run = rp.tile([P, TILE_F], f32)
                r3 = run[:].rearrange("p (j c) -> p j c", c=C)
                mb = m[:].unsqueeze(2).broadcast_to([P, GROUPS, C])
                nc.vector._custom_dve(
                    argmax_op, out=r3, in0=x3, in1=mb, s0=0.0, s1=PG_STEP)

                # idx = (8j + 14) - run[:, :, 7], cast to int32 on output
                o = op_.tile([P, GROUPS], i32)
                nc.vector.tensor_tensor(
                    out=o[:].unsqueeze(2), in0=i87[:].unsqueeze(2),
                    in1=r3[:, :, 7:8], op=Alu.subtract)
                nc.gpsimd.dma_start(out=yr[t], in_=o[:])
    nc.finalize()
    return nc


def _get_nc(n_tiles=N_TILES):
    if n_tiles not in _NC_CACHE:
        _NC_CACHE[n_tiles] = _build_nc(n_tiles)
    return _NC_CACHE[n_tiles]


def kernel(inputs, label_table):
    x = np.ascontiguousarray(np.asarray(inputs, dtype=np.float32))
    lt = np.asarray(label_table)
    assert x.shape == (ROWS, C), x.shape

    from concourse.bass_utils import run_bass_kernel_spmd

    nc = _get_nc()
    in_maps = [
        {"x": x[i * ROWS_PER_CORE:(i + 1) * ROWS_PER_CORE].reshape(-1)}
        for i in range(N_CORES)
    ]
    res = run_bass_kernel_spmd(nc, in_maps, core_ids=list(range(N_CORES)))
    idx = np.concatenate([res.results[i]["y"] for i in range(N_CORES)])
    return np.take(lt, idx).astype(lt.dtype)
